# revision 9
# baseline (speedup 1.0000x reference)
"""Trainium2 Bass kernel for the non-local (dot-product, no softmax) block.

Math: with x~ = [x_b; 1] (65 x N, ones row folds all conv biases), the whole
block collapses per batch to an affine map applied to x:

    f = theta^T phi / N ; y = f g  (associativity) =>
    z_b = x_b + A'_b x~_b,  A'^T_b = P1 S~_b P2 + E0

where S~_b = x~_b x~_b^T is the 65x65 raw Gram matrix of the augmented input,
P1 = theta~^T phi~ / N (65x65), P2 = g~^T rec_w^T (65x64),
E0 = [0_64x64; rec_b^T] (65x64), all host-precomputed from the conv weights.

Device work per batch: Gram accumulation over a host-pre-transposed fp8e4
copy of x (strided DoubleRow pairs (k, k+32) at 2 K-rows/cycle, 8-chunk
plain remainder), a 2-matmul fp32 sandwich to form A'^T, and a
(64x65)@(65xN) correction matmul emitted in fp16; x itself is re-added
exactly in fp32 on the host.

Sharding over 8 cores: cores 0-3 take batch 0, cores 4-7 batch 1. Each core
computes the full Gram for its batch (replicated; cross-core collectives
have a ~20us latency floor at this size) and produces one quarter of that
batch's output columns.

Pipelining: pools are hoisted and double-buffered so consecutive reps
overlap; each rep's sandwich + z-phase is emitted AFTER the next rep's
Gram, so the tensor engine streams Gram k+1 while rep k's sandwich chain
(PSUM->SBUF hops) resolves. consts load once; the Gram stream rides the SP
HWDGE ring, everything else the ACT ring.
"""

import ml_dtypes
import numpy as np

import concourse.bass as bass  # noqa: F401  (bass must import before bacc)
import concourse.bacc as bacc
import concourse.mybir as mybir
import concourse.tile as tile
from concourse.bass_utils import run_bass_kernel_spmd

B, C, HH, WW = 2, 64, 96, 96
N = HH * WW            # 9216
CA = C + 1             # 65: channels + ones row
NCORES = 8
GROUP = 4              # cores per batch
NS = N // GROUP        # 2304 output columns per core
KCH = N // 128         # 72 Gram chunks of 128
KDR = 32               # chunks 0..63 pair (k, k+32) via DoubleRow
ZCHUNK = 384           # z-phase matmul free dim (3 even chunks per half)
DT = mybir.dt.float32
DTB = mybir.dt.float8e4   # Gram operands: 1B/elem, DoubleRow 2 rows/cycle
DTH = mybir.dt.float16    # z-matmul operands + correction output
NPB = ml_dtypes.float8_e4m3

TRACE = False
LAST = None

_cached_nc = None


def _build(reps=1, loop_n=1):
    nc = bacc.Bacc(
        "TRN2",
        target_bir_lowering=False,
        debug=False,
        enable_asserts=False,
        num_devices=NCORES,
    )
    xnc_d = nc.dram_tensor("xnc", [128, KCH, CA], DTB, kind="ExternalInput")
    xnat_d = nc.dram_tensor("xnat", [CA, NS], DTH, kind="ExternalInput")
    consts_d = nc.dram_tensor("consts", [CA, CA + 2 * C], DT, kind="ExternalInput")
    zout_d = nc.dram_tensor("zout", [2, C, NS // 2], DTH, kind="ExternalOutput")

    with tile.TileContext(nc) as tc:
        with (
                tc.tile_pool(name="big", bufs=2) as big,
                tc.tile_pool(name="small", bufs=2) as small,
                tc.tile_pool(name="one", bufs=1) as one,
                tc.tile_pool(name="zs", bufs=3) as zsp,
                tc.tile_pool(name="ps", bufs=2, space="PSUM") as psp,
                tc.tile_pool(name="zps", bufs=2, space="PSUM") as zpsp,
        ):
            pools = (big, small, one, zsp, psp, zpsp)

            def body():
                state = {}
                for rep in range(reps):
                    _emit_head(nc, rep, pools, state, xnc_d, xnat_d, consts_d)
                    if rep > 0:
                        _emit_tail(nc, rep - 1, pools, state, zout_d)
                _emit_tail(nc, reps - 1, pools, state, zout_d)

            if loop_n > 1:
                # Hardware loop: reps-deep body repeated loop_n times on
                # device. Timing-only path (per-iteration all-engine
                # barrier); kernel() always uses loop_n=1.
                with tc.For_i(0, loop_n):
                    body()
            else:
                body()

    nc.compile()
    return nc


def _emit_head(nc, rep, pools, state, xnc_d, xnat_d, consts_d):
    big, small, one, zsp, psp, zpsp = pools
    if rep == 0:
        consts_t = one.tile([CA, CA + 2 * C], DT, tag="consts")
        nc.scalar.dma_start(consts_t[:], consts_d[:])
        state["consts"] = consts_t
    xnat_t = big.tile([CA, NS], DTH, tag="xnat")
    nc.scalar.dma_start(xnat_t[:], xnat_d[:])
    # Whole Gram stream in one 0.59MB transfer on the SP ring; double
    # buffering prefetches rep+1 while rep computes, hiding the latency.
    xnc_t = big.tile([128, KCH, CA], DTB, tag="xnc")
    nc.sync.dma_start(xnc_t[:], xnc_d[:])

    psS = psp.tile([CA, CA], DT, tag="S")
    if rep == 0:
        # PE warm-up: throwaway matmuls on a zeroed tile keep the tensor
        # engine busy during the initial DMA wait so the HAM clock gate is
        # already released when the Gram stream arrives. Reuses psS's bank.
        wz = small.tile([128, C], DTB, tag="wz")
        nc.vector.memset(wz[:], 0)
        # tiny activation-copy loads the ACT function table off the
        # critical path, so the z-phase ACT copies run warm
        aw = small.tile([1, 1], DT, tag="aw")
        nc.scalar.copy(aw[:], wz[0:1, 0:4].bitcast(DT))
        for _ in range(12):
            nc.tensor.matmul(psS[0:C, 0:C], wz[:], wz[:], start=True, stop=True)

    # Gram: S~ += chunk^T @ chunk over all 72 chunks. Chunks 0..63 run as
    # strided DoubleRow pairs (k, k+32) -- the 2080B k-pair stride satisfies
    # the 16B-aligned LDWEIGHTS constraint without padding; the last 8 run
    # plain at 1 K-row/cycle.
    for k in range(KDR):
        ap = xnc_t[:, k:k + KDR + 1:KDR, :]
        nc.tensor.matmul(
            psS[:], ap, ap,
            start=(k == 0), stop=False,
            perf_mode=mybir.MatmulPerfMode.DoubleRow,
        )
    for k in range(2 * KDR, KCH):
        ap = xnc_t[:, k, :]
        nc.tensor.matmul(psS[:], ap, ap, start=False, stop=(k == KCH - 1))
    sS = small.tile([CA, CA], DT, tag="sS")
    nc.vector.tensor_copy(sS[:], psS[:])

    state[rep] = (xnat_t, psS, sS)


def _emit_tail(nc, rep, pools, state, zout_d):
    big, small, one, zsp, psp, zpsp = pools
    xnat_t, psS, sS = state.pop(rep)
    consts_t = state["consts"]
    p1t_t = consts_t[:, 0:CA]
    p2_t = consts_t[:, CA:CA + C]
    e0_t = consts_t[:, CA + C:CA + 2 * C]

    # A'^T = P1 @ (S~ @ P2) + E0   (S~ symmetric, so lhsT = S~ works).
    # V and W reuse psS's PSUM bank -- S was already drained to sS.
    psV = psS[0:CA, 0:C]
    nc.tensor.matmul(psV, sS[:], p2_t, start=True, stop=True)
    sV = small.tile([CA, C], DT, tag="sV")
    nc.vector.tensor_copy(sV[:], psV)
    nc.tensor.matmul(psV, p1t_t, sV[:], start=True, stop=True)
    sAT = small.tile([CA, C], DTH, tag="sAT")
    nc.vector.tensor_add(sAT[:], psV, e0_t)

    # z slice = A' @ x~ (the correction only; x is re-added on the host),
    # in folded column pairs: the matmuls for columns n and n+NS/2 write
    # the top/bottom partition halves of ONE PSUM bank, so PSUM drains run
    # at full 128-lane width, zout DMAs span 128 partitions, and the
    # z-phase only holds 2 banks total.
    half = NS // 2
    off = 0
    while off < half:
        w = min(ZCHUNK, half - off)
        pz = zpsp.tile([128, ZCHUNK], DT, tag="pz")
        nc.tensor.matmul(
            pz[0:C, :w], sAT[:], xnat_t[:, off:off + w],
            start=True, stop=True,
        )
        nc.tensor.matmul(
            pz[C:128, :w], sAT[:], xnat_t[:, half + off:half + off + w],
            start=True, stop=True, tile_position=(0, C),
        )
        # PSUM->SBUF drains split across DVE and ACT so the two halves run
        # in parallel; fp16 halves both the copy and the store bytes.
        zt = zsp.tile([128, ZCHUNK], DTH, tag="zt")
        nc.vector.tensor_copy(zt[0:C, :w], pz[0:C, :w])
        nc.scalar.copy(zt[C:128, :w], pz[C:128, :w])
        nc.scalar.dma_start(zout_d[:, :, off:off + w], zt[:, :w])
        off += w


def _host_prep(x, theta_w, theta_b, phi_w, phi_b, g_w, g_b, rec_w, rec_b):
    f8 = np.float64
    ta = np.concatenate([theta_w, theta_b[:, None]], 1).astype(f8)  # (32, 65)
    pa = np.concatenate([phi_w, phi_b[:, None]], 1).astype(f8)
    ga = np.concatenate([g_w, g_b[:, None]], 1).astype(f8)
    p1t = (pa.T @ ta / N).astype(np.float32)  # (65, 65)
    p2 = (ga.T @ rec_w.astype(f8).T).astype(np.float32)
    e0 = np.zeros((CA, C), np.float32)
    e0[C, :] = rec_b.astype(np.float32)
    consts = np.ascontiguousarray(np.concatenate([p1t, p2, e0], axis=1))

    in_maps = []
    xncs, xnats = [], []
    for b in range(B):
        xb = np.ascontiguousarray(x[b].reshape(C, N), dtype=np.float32)
        xt = np.concatenate([xb, np.ones((1, N), np.float32)], 0)  # (65, N)
        # xnc[p, k, c] = x~[c, 128k+p]: each (128, CA) chunk is directly a
        # K=128 matmul operand; layout is the SBUF image, so DMA is trivial.
        xnc = np.ascontiguousarray(
            xt.reshape(CA, KCH, 128).transpose(2, 1, 0).astype(NPB)
        )
        xncs.append(xnc)
        xnats.append(xt)
    for c in range(NCORES):
        b, q = divmod(c, GROUP)
        in_maps.append(
            {
                "xnc": xncs[b],
                "xnat": np.ascontiguousarray(
                    xnats[b][:, q * NS:(q + 1) * NS].astype(np.float16)
                ),
                "consts": consts,
            }
        )
    return in_maps


def kernel(x, theta_w, theta_b, phi_w, phi_b, g_w, g_b, rec_w, rec_b):
    global _cached_nc, LAST
    x = np.asarray(x)
    theta_w, theta_b = np.asarray(theta_w), np.asarray(theta_b)
    phi_w, phi_b = np.asarray(phi_w), np.asarray(phi_b)
    g_w, g_b = np.asarray(g_w), np.asarray(g_b)
    rec_w, rec_b = np.asarray(rec_w), np.asarray(rec_b)
    if _cached_nc is None:
        _cached_nc = _build()
    in_maps = _host_prep(
        x, theta_w, theta_b, phi_w, phi_b, g_w, g_b, rec_w, rec_b
    )
    LAST = run_bass_kernel_spmd(
        _cached_nc, in_maps, list(range(NCORES)), trace=TRACE
    )
    z = np.empty((B, C, N), np.float32)
    for c in range(NCORES):
        b, q = divmod(c, GROUP)
        zo = LAST.results[c]["zout"]  # (2, C, NS//2) folded correction halves
        z[b][:, q * NS:q * NS + NS // 2] = zo[0]
        z[b][:, q * NS + NS // 2:(q + 1) * NS] = zo[1]
    z += x.reshape(B, C, N)  # exact fp32 passthrough, added host-side
    return z.reshape(B, C, HH, WW)
